# revision 15
# baseline (speedup 1.0000x reference)
"""Trainium2 Bass kernel for nn_CrossDenseLayer (moe_routing).

Computes out[b,t,n,v,m,j] = sum_i x[b,t,n,v,m,i] * weights[emb_var[b,v], i, m, j]

Shapes (hardcoded, from the problem spec):
  x:       [4, 32, 64, 8, 8, 128] fp32   (256 MB)
  weights: [32, 128, 8, 16]       fp32   (2 MB)
  emb_var: [4, 8]                 int    (routing indices)
  out:     [4, 32, 64, 8, 8, 16]  fp32   (32 MB)

Sharding: data-parallel over (b, t-half) -> 8 shards. Core c handles
b = c//2, t in [16*(c%2), 16*(c%2)+16). The per-(b,v) weight gather is
done on host (it is a 512 KB slice); each core receives its own
gathered weight bank w[v, i, m, j] = weights[emb_var[b, v]].

Per-core device kernel: for each 128-row tile of the 1024 (t,n) rows:
  - DMA the full [128 rows, (v m i)=8192] slab (4 MB, contiguous)
  - for each of the 64 (v,m) pairs: PE-transpose the [128 rows, 128 i]
    subtile into PSUM, copy to SBUF (ACT/DVE alternate), then
    matmul(lhsT=xT[i,rows], rhs=w[i, 16 j]) -> PSUM out[rows, 16]
    packed side by side so the row-tile's output [128, (v m j)=1024]
    is contiguous
  - copy PSUM->SBUF and DMA out (512 KB, contiguous)
"""

import sys

import numpy as np

try:
    import concourse  # noqa: F401
except ImportError:  # fallback if PYTHONPATH doesn't carry the repo
    for _p in ("/opt/trn_rl_repo", "/root/.axon_site/_ro/trn_rl_repo"):
        if _p not in sys.path:
            sys.path.insert(0, _p)

B, T, N, V, F, FI, J = 4, 32, 64, 8, 8, 128, 16
NCORES = 8
TS = T // 2          # t rows per shard = 16
ROWS = TS * N        # 1024 rows per core
VM = V * F           # 64
XF = VM * FI         # 8192 floats per row of x
OF = VM * J          # 1024 floats per row of out
RT = ROWS // 128     # 8 row tiles

_CACHE = {}


def _build_kernel():
    import concourse.bass as bass
    import concourse.bacc as bacc
    import concourse.tile as tile
    from concourse import mybir
    from concourse.masks import make_identity
    from contextlib import ExitStack

    fp32 = mybir.dt.float32
    nc = bacc.Bacc("TRN2", target_bir_lowering=False, debug=False,
                   num_devices=NCORES)
    x_d = nc.dram_tensor("x", [ROWS, XF], fp32, kind="ExternalInput").ap()
    w_d = nc.dram_tensor("w", [V, FI, F * J], fp32, kind="ExternalInput").ap()
    o_d = nc.dram_tensor("o", [ROWS, OF], fp32, kind="ExternalOutput").ap()

    with tile.TileContext(nc) as tc, ExitStack() as ctx:
        const = ctx.enter_context(tc.tile_pool(name="const", bufs=1))
        xpool = ctx.enter_context(tc.tile_pool(name="xin", bufs=5))
        xtp = ctx.enter_context(tc.tile_pool(name="xt", bufs=4))
        osb_p = ctx.enter_context(tc.tile_pool(name="osb", bufs=4))
        pst = ctx.enter_context(tc.tile_pool(name="pst", bufs=4, space="PSUM"))
        pso = ctx.enter_context(tc.tile_pool(name="pso", bufs=4, space="PSUM"))

        ident = const.tile([128, 128], fp32)
        make_identity(nc, ident)
        # weights in SBUF as [i=128 partitions, v, (m j)=128]
        wsb = const.tile([FI, V, F * J], fp32)
        # weight DMA on the ACT queue so it doesn't delay the x stream on SP
        nc.scalar.dma_start(out=wsb[:], in_=w_d.rearrange("v i mj -> i v mj"))

        # PE warm-up ops: each carries exactly ONE semaphore wait so that
        # transpose-mode matmuls (single LDW-struct instr, one wait slot)
        # in the main loop never need more than one fresh wait.
        warm = pst.tile([128, 512], fp32, tag="ps")
        nc.tensor.transpose(warm[:, :128], ident[:], ident[:])  # waits: Pool
        nc.tensor.matmul(warm[:, 128:144], lhsT=ident[:],
                         rhs=wsb[:, 0, :16], start=True, stop=True)  # waits: wsb DMA

        HF = XF // 2                     # half row-slab free size (v 0-3 / 4-7)
        for r in range(RT):
            # two half-slabs per row tile; input DMAs stay on the SP
            # (sync) HWDGE queue so they flow back-to-back. First row tile
            # is loaded in quarters so the PE pipeline fills sooner.
            xh = []
            for h in range(2):
                xt_in = xpool.tile([128, HF], fp32, tag="xin")
                if r == 0:
                    nq = 8 if h == 0 else 2
                    QF = HF // nq
                    for q in range(nq):
                        nc.sync.dma_start(
                            out=xt_in[:, q * QF:(q + 1) * QF],
                            in_=x_d[r * 128:(r + 1) * 128,
                                    h * HF + q * QF:h * HF + (q + 1) * QF])
                else:
                    nc.sync.dma_start(
                        out=xt_in[:],
                        in_=x_d[r * 128:(r + 1) * 128, h * HF:(h + 1) * HF])
                xh.append(xt_in)
            ops0 = pso.tile([128, 512], fp32, tag="ops")
            ops1 = pso.tile([128, 512], fp32, tag="ops")
            opsl = (ops0, ops1)
            for p in range(16):          # packs of 4 (v,m) pairs
                xin = xh[p // 8]
                ps = pst.tile([128, 512], fp32)
                for k in range(4):
                    vm = p * 4 + k
                    col = vm * 128 - (p // 8) * HF
                    nc.tensor.transpose(
                        ps[:, k * 128:(k + 1) * 128],
                        xin[:, col:col + 128],
                        ident[:],
                    )
                xt = xtp.tile([128, 512], fp32)
                if p % 2 == 0:
                    nc.scalar.copy(out=xt[:], in_=ps[:])
                else:
                    nc.vector.tensor_copy(out=xt[:], in_=ps[:])
                for k in range(4):
                    vm = p * 4 + k
                    v, m = divmod(vm, F)
                    nc.tensor.matmul(
                        opsl[vm // 32][:, (vm % 32) * 16:(vm % 32 + 1) * 16],
                        lhsT=xt[:, k * 128:(k + 1) * 128],
                        rhs=wsb[:, v, m * 16:(m + 1) * 16],
                        start=True, stop=True,
                    )
                # drain each half of the output as soon as it completes
                # (pack 7 finishes ops0, pack 15 finishes ops1); output
                # DMAs ride the ACT HWDGE queue so they never block the
                # input stream on SP
                if p == 7:
                    osb0 = osb_p.tile([128, 512], fp32, tag="osb")
                    nc.scalar.copy(out=osb0[:], in_=ops0[:])
                    nc.scalar.dma_start(
                        out=o_d[r * 128:(r + 1) * 128, :512], in_=osb0[:])
                elif p == 11 and r == RT - 1:
                    # last row tile: drain ops1's first half early to
                    # shorten the kernel tail
                    osb1a = osb_p.tile([128, 512], fp32, tag="osb")
                    nc.vector.tensor_copy(out=osb1a[:, :256], in_=ops1[:, :256])
                    nc.scalar.dma_start(
                        out=o_d[r * 128:(r + 1) * 128, 512:768],
                        in_=osb1a[:, :256])
                elif p == 15:
                    if r == RT - 1:
                        osb1 = osb_p.tile([128, 512], fp32, tag="osb")
                        nc.vector.tensor_copy(out=osb1[:, :256],
                                              in_=ops1[:, 256:])
                        nc.scalar.dma_start(
                            out=o_d[r * 128:(r + 1) * 128, 768:],
                            in_=osb1[:, :256])
                    else:
                        osb1 = osb_p.tile([128, 512], fp32, tag="osb")
                        nc.vector.tensor_copy(out=osb1[:], in_=ops1[:])
                        nc.scalar.dma_start(
                            out=o_d[r * 128:(r + 1) * 128, 512:], in_=osb1[:])
    nc.finalize()
    return nc


def _shard_inputs(x, weights, emb_var):
    x = np.asarray(x, dtype=np.float32)
    weights = np.asarray(weights, dtype=np.float32)
    ev = np.asarray(emb_var).astype(np.int64)
    in_maps = []
    for c in range(NCORES):
        b, th = divmod(c, 2)
        xs = np.ascontiguousarray(
            x[b, th * TS:(th + 1) * TS]).reshape(ROWS, XF)
        ws = np.ascontiguousarray(
            weights[ev[b]]).reshape(V, FI, F * J)
        in_maps.append({"x": xs, "w": ws})
    return in_maps


def kernel(x, weights, emb_var, **_unused):
    from concourse.bass_utils import run_bass_kernel_spmd

    if "nc" not in _CACHE:
        _CACHE["nc"] = _build_kernel()
    nc = _CACHE["nc"]

    in_maps = _shard_inputs(x, weights, emb_var)
    res = run_bass_kernel_spmd(nc, in_maps, list(range(NCORES))).results

    out = np.empty((B, T, N, V, F, J), np.float32)
    for c in range(NCORES):
        b, th = divmod(c, 2)
        out[b, th * TS:(th + 1) * TS] = res[c]["o"].reshape(TS, N, V, F, J)
    return out
